# revision 33
# baseline (speedup 1.0000x reference)
"""Trainium2 Bass kernel for nn_CODEXReconstruction (moe_routing).

Data-parallel over the batch across 8 NeuronCores; all weights replicated.
Activations live transposed ([features, batch]) so every layer's weight is
the matmul stationary operand. Per-core (B=1024):

    enc1:  h1  = relu(W1.T @ xT + b1)              [512, 1024]  bf16
    enc2:  emb = relu(W2.T @ h1 + b2)              [256, 1024]  bf16
    experts (host-routed):
        The host sorts each core's columns by the sample's PRIMARY
        treatment and balances counts so every core has exactly n1[t]
        primary-t columns (leftovers + no-treatment samples form the
        tail). Pass 1 applies expert t only to its contiguous primary
        range. Remaining (sample, treatment) slots go through pass 2:
        one-hot GATHER (emb columns -> treatment-sorted slots, via PE
        matmul against host-built P), per-treatment expert matmuls in
        transposed orientation (stationary = gathered emb, moving =
        T_W rows -> out2T[slot, e']), then one-hot SCATTER-add back to
        sample columns (stationary = out2T, moving = Q). This does
        ~1.9 expert applications per sample instead of 20.
    dec1/dec2: relu matmuls                        [512, 1024]  bf16
    dec3: vars half (rows >= 5000 pre-softplus) in bf16 -- it carries
        97.7% of the output L2 norm. Means half in fp8e4 DoubleRow
        (two k-tiles per instruction = 2x PE throughput); means carry
        2.3% of the norm so fp8 quantization error is negligible.
        softplus+0.001 = ln(C + C*e^x) via EXP -> bf16 TS -> LN.
        Vars pairs are front-loaded so the kernel tail drains the cheap
        means epilogue (split ACT/DVE, 512-col chunks), not EXP/LN.

bf16 keeps the PE's HAM clock gate warm; keep-warm dummy matmuls bridge
the one unavoidable epilogue-only window. Outputs are written fp16.
Weight tiles are pre-packed on host so every DMA moves >=1KB per
partition line; constants ride the GpSimd queue; P/Q ride the Sync and
Act queues after the enc1 stream. The routing (incl. the full-batch
apply_t rule) is computed on host from the integer treatment tensor;
per-treatment counts are baked into the program at build time.
"""

import numpy as np
import ml_dtypes

import bass_rust
import concourse.bass as bass
import concourse.mybir as mybir
import concourse.tile as tile
from concourse.bass_utils import run_bass_kernel_spmd
from concourse.tile import ScopedClock

# ---------------------------------------------------------------------------
# Problem constants (hardcoded per contract)
# ---------------------------------------------------------------------------
IN_F = 5000
IN_FP = 5120                  # zero-padded K so k-tiles are uniform 128
N0, N1, N2 = 512, 512, 256
T = 20
BATCH = 8192
N_CORES = 8
B = BATCH // N_CORES          # 1024 per core
NB = B // 512                 # moving-dim chunks of 512
KP = IN_FP // 256             # 20 packed x/w1 stream steps (2 k-tiles each)
MT_HALF = 40                  # 5000 out-features -> 40 m-tiles (last 8 valid)

F32 = mybir.dt.float32
F16 = mybir.dt.float16
BF16 = mybir.dt.bfloat16
F8 = mybir.dt.float8e4
DOUBLE_ROW = mybir.MatmulPerfMode.DoubleRow
W3M_SCALE = 128.0  # dec_W3 means half pre-scaled into fp8e4 normal range
RELU = mybir.ActivationFunctionType.Relu
IDENT = mybir.ActivationFunctionType.Identity
EXP = mybir.ActivationFunctionType.Exp
LN = mybir.ActivationFunctionType.Ln
ADD = mybir.AluOpType.add
MULT = mybir.AluOpType.mult
MAX = mybir.AluOpType.max
# softplus(x)+0.001 = ln(C + C*e^x) with C = e^0.001 (this walrus build has no
# Softplus act table; exp/ln/relu/identity all live in one table set)
SP_C = 1.0010005001667084

# ---------------------------------------------------------------------------
# Workaround: this walrus build rejects >1 sync wait per instruction.
# Split extra waits onto injected same-engine NoOps (engine streams are
# in-order, so a preceding same-engine wait is equivalent), and chunk the
# Tile tail-drain's waits across chained drain instructions.
# ---------------------------------------------------------------------------
_uid = [0]


def _nop_with_wait(engine, wait):
    _uid[0] += 1
    nop = mybir.InstNoOp(name=f"WSPLIT-{_uid[0]}", ins=[], outs=[])
    nop.engine = engine
    nop.sync_info = bass_rust.SyncInfo(on_wait=[wait], on_update=[])
    return nop


def split_sync_waits(nc):
    for f in nc.m.functions:
        for bb in f.blocks:
            old = bb.instructions
            if not any(
                i.sync_info and i.sync_info.on_wait and len(i.sync_info.on_wait) > 1
                for i in old
            ):
                continue
            new = []
            for inst in old:
                si = inst.sync_info
                if si is not None and si.on_wait and len(si.on_wait) > 1:
                    waits = list(si.on_wait)
                    for w in waits[:-1]:
                        new.append(_nop_with_wait(inst.engine, w))
                    si.on_wait = [waits[-1]]
                new.append(inst)
            bb.instructions = new


def _patched_drain_and_barrier(self, tick_clock, wait_clock):
    nc = self.nc
    drain_inst = nc.sync.drain()
    wait_clock.add_sem_waits(
        drain_inst.ins, ScopedClock({None: tick_clock.global_clock})
    )
    waits = list(drain_inst.ins.sync_info.on_wait or [])
    if len(waits) > 1:
        drain_inst.ins.sync_info.on_wait = waits[:1]
        for i in range(1, len(waits)):
            extra = nc.sync.drain()
            if extra.ins.sync_info is None:
                extra.ins.sync_info = bass_rust.SyncInfo(
                    on_wait=[waits[i]], on_update=[]
                )
            else:
                extra.ins.sync_info.on_wait = [waits[i]]

    nc.all_engine_barrier()
    assert self.sems is not None
    popped = nc._tile_sem_poison_stack.pop()
    assert popped is self._sem_poison
    nc.clear_and_free_semaphores(list(self.sems.allocated().values()))
    nc.all_engine_barrier()


tile.TileContext._drain_and_barrier = _patched_drain_and_barrier


def _chunks512(lo, hi):
    """Split [lo, hi) at absolute multiples of 512 (PSUM bank boundaries)."""
    out = []
    a = lo
    while a < hi:
        b = min(hi, (a // 512 + 1) * 512)
        out.append((a, b))
        a = b
    return out


# ---------------------------------------------------------------------------
# Host-side routing: primary/secondary assignment, core balancing, P/Q.
# ---------------------------------------------------------------------------
class Route:
    pass


def _route(inputs):
    treat = np.asarray(inputs["treatment"])
    tvals = np.arange(1, T + 1)
    mask = (treat[:, None, :] == tvals[None, :, None]).any(-1)  # [8192, T]
    apply_t = mask.sum(0) > 1
    gate = mask & apply_t[None, :]

    prim = np.full(BATCH, -1, np.int64)
    sec = np.full(BATCH, -1, np.int64)
    pair_flip = {}
    gate_lists = [np.flatnonzero(gate[i]) for i in range(BATCH)]
    for i in range(BATCH):
        ts = gate_lists[i]
        if len(ts) == 1:
            prim[i] = ts[0]
        elif len(ts) == 2:
            a, b = int(ts[0]), int(ts[1])
            f = pair_flip.get((a, b), 0)
            pair_flip[(a, b)] = 1 - f
            prim[i], sec[i] = (a, b) if f == 0 else (b, a)

    n1 = np.array([(prim == t).sum() // N_CORES for t in range(T)], np.int64)

    core_of = np.full(BATCH, -1, np.int64)
    in_p1 = np.zeros(BATCH, bool)
    quota = np.tile(n1[None, :], (N_CORES, 1)).copy()
    c2 = np.zeros((N_CORES, T), np.int64)
    load = np.zeros(N_CORES, np.int64)
    tail = [i for i in range(BATCH) if prim[i] < 0]
    for t in range(T):
        for i in np.flatnonzero(prim == t):
            cand = [c for c in range(N_CORES) if quota[c, t] > 0]
            if not cand:
                tail.append(i)
                continue
            s = sec[i]
            if s >= 0:
                c = min(cand, key=lambda c: (c2[c, s], load[c], c))
            else:
                c = min(cand, key=lambda c: (load[c], c))
            quota[c, t] -= 1
            core_of[i] = c
            in_p1[i] = True
            load[c] += 1
            if s >= 0:
                c2[c, s] += 1
    cap_tail = B - int(n1.sum())
    tcount = np.zeros(N_CORES, np.int64)
    for i in tail:
        ts = gate_lists[i]
        cand = [c for c in range(N_CORES) if tcount[c] < cap_tail]
        c = min(
            cand,
            key=lambda c: (
                max((c2[c, t] for t in ts), default=0), tcount[c], c
            ),
        )
        core_of[i] = c
        tcount[c] += 1
        load[c] += 1
        for t in ts:
            c2[c, t] += 1
    assert np.all(load == B)

    cap2 = c2.max(axis=0)
    O = np.zeros(T + 1, np.int64)
    for t in range(T):
        O[t + 1] = O[t] + cap2[t]
    S2 = int(O[T])
    NS2 = (S2 + 127) // 128
    S2P = NS2 * 128
    assert S2P <= 1024, f"secondary slot space {S2P} exceeds 1024"

    perm = np.zeros((N_CORES, B), np.int64)
    P = np.zeros((N_CORES, B, S2P), np.float32)
    Q = np.zeros((N_CORES, S2P, B), np.float32)
    for c in range(N_CORES):
        cols = []
        for t in range(T):
            members = np.flatnonzero((core_of == c) & (prim == t) & in_p1)
            assert len(members) == n1[t]
            cols.extend(members.tolist())
        cols.extend(np.flatnonzero((core_of == c) & ~in_p1).tolist())
        assert len(cols) == B
        perm[c] = cols
        used = np.zeros(T, np.int64)
        for local_b, gi in enumerate(cols):
            if in_p1[gi]:
                slots = [sec[gi]] if sec[gi] >= 0 else []
            else:
                slots = gate_lists[gi].tolist()
            for t in slots:
                sl = O[t] + used[t]
                used[t] += 1
                P[c, local_b, sl] = 1.0
                Q[c, sl, local_b] = 1.0
        assert np.all(used <= cap2)

    r = Route()
    r.n1 = tuple(int(v) for v in n1)
    r.cap2 = tuple(int(v) for v in cap2)
    r.O = tuple(int(v) for v in O)
    r.S2 = S2
    r.NS2 = NS2
    r.S2P = S2P
    r.n_act = int(n1.sum())
    r.tb_zero = not np.any(np.asarray(inputs["T_b"]))
    r.perm = perm
    r.P = P
    r.Q = Q
    r.meta = (r.n1, r.cap2, r.O, r.S2, r.NS2, r.S2P, r.n_act, r.tb_zero)
    return r


# ---------------------------------------------------------------------------
# Bass module (one NeuronCore's program; SPMD across 8 cores)
# ---------------------------------------------------------------------------
def build_bass(meta):
    n1, cap2, O, S2, NS2, S2P, n_act, tb_zero = meta
    nc = bass.Bass()

    # packed streams: per step j, x holds k-tiles 2j,2j+1 side by side
    xp = nc.dram_tensor("xp", [KP, 128, 2 * B], BF16, kind="ExternalInput")
    w1p = nc.dram_tensor("w1p", [KP, 128, 2 * N0], BF16, kind="ExternalInput")
    w2 = nc.dram_tensor("w2", [N0, N2], BF16, kind="ExternalInput")
    twp = nc.dram_tensor("twp", [T, 128, 2 * N2], BF16, kind="ExternalInput")
    dw1 = nc.dram_tensor("dw1", [N2, N1], BF16, kind="ExternalInput")
    dw2 = nc.dram_tensor("dw2", [N1, N0], BF16, kind="ExternalInput")
    # vars half (bf16): w3v[j, p, mi2*512 + k*128 + c] = W3[k*128+p, (2j+mi2)*128+c]
    w3v = nc.dram_tensor("w3v", [MT_HALF // 2, 128, 1024], BF16, kind="ExternalInput")
    # means half (fp8e4, x128): DoubleRow k-pair planes:
    # w3m8[j, p, mi2*512 + kk*256 + pl*128 + c] = W3[(2kk+pl)*128+p, (2j+mi2)*128+c]
    w3m8 = nc.dram_tensor("w3m8", [MT_HALF // 2, 128, 1024], F8, kind="ExternalInput")
    # routing one-hots: P[b, slot] (gather), Q[slot, b] (scatter)
    pd = nc.dram_tensor("pd", [8, 128, S2P], BF16, kind="ExternalInput")
    qd = nc.dram_tensor("qd", [NS2, 128, B], BF16, kind="ExternalInput")
    idm = nc.dram_tensor("idm", [128, 128], BF16, kind="ExternalInput")
    tbr = nc.dram_tensor("tbr", [1, T * N2], BF16, kind="ExternalInput")
    # bias columns: [128, n_tiles], col j = bias[j*128 : (j+1)*128]
    b1c = nc.dram_tensor("b1c", [128, 4], F32, kind="ExternalInput")
    b2c = nc.dram_tensor("b2c", [128, 2], F32, kind="ExternalInput")
    tbc = nc.dram_tensor("tbc", [128, T * 2], F32, kind="ExternalInput")
    db1c = nc.dram_tensor("db1c", [128, 4], F32, kind="ExternalInput")
    db2c = nc.dram_tensor("db2c", [128, 4], F32, kind="ExternalInput")
    b3mc = nc.dram_tensor("b3mc", [128, MT_HALF], F32, kind="ExternalInput")
    b3vc = nc.dram_tensor("b3vc", [128, MT_HALF], F32, kind="ExternalInput")

    yt = nc.dram_tensor("yt", [2 * IN_F, B], F16, kind="ExternalOutput")

    with tile.TileContext(nc) as tc:
        with (
            tc.tile_pool(name="const", bufs=1) as const,
            tc.tile_pool(name="acts", bufs=8) as acts,
            tc.tile_pool(name="xpr", bufs=1) as xpr,
            tc.tile_pool(name="xs", bufs=8) as xs,
            tc.tile_pool(name="ws", bufs=6) as wsp,
            tc.tile_pool(name="tws", bufs=T) as tws,
            tc.tile_pool(name="w3s", bufs=4) as w3s,
            tc.tile_pool(name="w3s8", bufs=4) as w3s8,
            tc.tile_pool(name="outs", bufs=3) as outs,
            tc.tile_pool(name="rp", bufs=3) as rp,
            tc.tile_pool(name="ps", bufs=4, space="PSUM") as psp,
        ):
            # ------- persistent constants (GpSimd queue, off the load path)
            id_sb = const.tile([128, 128], BF16, name="id_sb")
            nc.gpsimd.dma_start(out=id_sb[:], in_=idm[:])
            w2_sb = []
            for k in range(4):
                t_ = const.tile([128, N2], BF16, name=f"w2_{k}")
                nc.gpsimd.dma_start(out=t_[:], in_=w2[k * 128:(k + 1) * 128, :])
                w2_sb.append(t_)

            # ------- HAM warm-up: ~10 dummy matmuls on a DVE-memset tile
            # run during the initial DMA latency so the clock gate is at 8/8
            # when enc1 starts (DMA'd constants arrive too late for this)
            warm = const.tile([128, 512], BF16, name="warm")
            nc.vector.memset(warm[:], 0.0)
            nc.scalar.activation(warm[0:1, 1:2], warm[0:1, 0:1], RELU, bias=warm[0:1, 0:1])
            wps = psp.tile([128, 512], F32, name="wps", tag="ps")
            for i in range(10):
                nc.tensor.matmul(
                    wps[:], warm[:, :128], warm[:], start=(i == 0), stop=(i == 9)
                )

            def keep_warm(n):
                # LDWEIGHTS-only filler: streams rows through the PE array
                # (keeps the HAM clock gate + p-state up during epilogue-only
                # windows) without touching PSUM or waiting on anything
                for _ in range(n):
                    nc.tensor.ldweights(warm[:, :128])
            dw1_sb = []
            for k in range(2):
                t_ = const.tile([128, N1], BF16, name=f"dw1_{k}")
                nc.gpsimd.dma_start(out=t_[:], in_=dw1[k * 128:(k + 1) * 128, :])
                dw1_sb.append(t_)
            dw2_sb = []
            for k in range(4):
                t_ = const.tile([128, N0], BF16, name=f"dw2_{k}")
                nc.gpsimd.dma_start(out=t_[:], in_=dw2[k * 128:(k + 1) * 128, :])
                dw2_sb.append(t_)
            tbr_sb = None
            if not tb_zero:
                tbr_sb = const.tile([1, T * N2], BF16, name="tbr_sb")
                nc.gpsimd.dma_start(out=tbr_sb[:], in_=tbr[:])
                ones_sb = const.tile([1, 512], BF16, name="ones_sb")
                nc.vector.memset(ones_sb[:], 1.0)

            def load_bias(name, src, cols):
                t_ = const.tile([128, cols], F32, name=name)
                nc.gpsimd.dma_start(out=t_[:], in_=src[:])
                return t_

            b1_sb = load_bias("b1_sb", b1c, 4)
            b2_sb = load_bias("b2_sb", b2c, 2)
            db1_sb = load_bias("db1_sb", db1c, 4)
            db2_sb = load_bias("db2_sb", db2c, 4)
            b3m_sb = load_bias("b3m_sb", b3mc, MT_HALF)
            b3v_sb = load_bias("b3v_sb", b3vc, MT_HALF)

            def mk_psum(tag_name):
                # [128, 1024] = 2 PSUM banks; matmuls fill 512-wide halves
                return psp.tile([128, B], F32, name=tag_name, tag="ps")

            # ------- enc1 (bf16): [5120,1024] -> [512,1024]
            h1 = [
                acts.tile([128, B], BF16, name=f"h1_{m}", tag="a1024")
                for m in range(4)
            ]
            ps_h1 = [mk_psum(f"psh1_{m}") for m in range(4)]
            xk_list = []
            for j in range(KP):
                xk = xs.tile([128, 2 * B], BF16, name=f"x_{j}", tag="x")
                xk_list.append(xk)
                w1k = wsp.tile([128, 2 * N0], BF16, name=f"w1_{j}", tag="w")
                if j == 0:
                    # halves so the s=0 matmuls start after ~half the bytes
                    nc.scalar.dma_start(out=w1k[:, :N0], in_=w1p[j, :, :N0])
                    nc.sync.dma_start(out=xk[:, :B], in_=xp[j, :, :B])
                    nc.scalar.dma_start(out=w1k[:, N0:], in_=w1p[j, :, N0:])
                    nc.sync.dma_start(out=xk[:, B:], in_=xp[j, :, B:])
                else:
                    nc.sync.dma_start(out=xk[:], in_=xp[j])
                    nc.scalar.dma_start(out=w1k[:], in_=w1p[j])
                for s in range(2):
                    for m in range(4):
                        for n in range(NB):
                            nc.tensor.matmul(
                                ps_h1[m][:, n * 512:(n + 1) * 512],
                                w1k[:, s * N0 + m * 128: s * N0 + (m + 1) * 128],
                                xk[:, s * B + n * 512: s * B + (n + 1) * 512],
                                start=(j == 0 and s == 0),
                                stop=(j == KP - 1 and s == 1),
                            )
            # expert weights + routing one-hots: emitted after the enc1
            # stream so they don't contend with it; the GpSimd, Sync and Act
            # DMA queues are all idle from here until the dec3 w3 loads /
            # output stores
            gate16 = xk_list[KP - 1]
            gate14 = xk_list[KP - 1]
            tw_sb = []
            for t in range(T):
                t_ = tws.tile([128, 2 * N2], BF16, name=f"tw_{t}", tag="tw")
                nc.gpsimd.tensor_copy(t_[0:1, 0:1], gate14[0:1, 0:1])
                nc.gpsimd.dma_start(out=t_[:], in_=twp[t])
                tw_sb.append(t_)
            p_sb = []
            for jb in range(8):
                t_ = const.tile([128, S2P], BF16, name=f"p_{jb}")
                nc.gpsimd.tensor_copy(t_[0:1, 0:1], gate16[0:1, 0:1])
                nc.sync.dma_start(out=t_[:], in_=pd[jb])
                p_sb.append(t_)
            q_sb = []
            for j2 in range(NS2):
                t_ = const.tile([128, B], BF16, name=f"q_{j2}")
                nc.gpsimd.tensor_copy(t_[0:1, 0:1], gate16[0:1, 0:1])
                nc.scalar.dma_start(out=t_[:], in_=qd[j2])
                q_sb.append(t_)

            # h1 epilogue split across ACT and DVE so the serial relu chain
            # (which gates enc2's PSUM slot reuse) halves; PE stays warm on
            # LDWEIGHTS filler meanwhile
            for m, n in [(0, 0), (1, 0), (2, 1), (3, 1)]:
                sl = slice(n * 512, (n + 1) * 512)
                nc.scalar.activation(
                    h1[m][:, sl], ps_h1[m][:, sl], RELU, bias=b1_sb[:, m:m + 1]
                )
            for m, n in [(0, 1), (1, 1), (2, 0), (3, 0)]:
                sl = slice(n * 512, (n + 1) * 512)
                nc.vector.tensor_scalar(
                    h1[m][:, sl], ps_h1[m][:, sl], b1_sb[:, m:m + 1], 0.0,
                    op0=ADD, op1=MAX,
                )

            # ------- enc2 (bf16): [512,1024] -> [256,1024]
            emb = [
                acts.tile([128, B], BF16, name=f"emb_{m}", tag="a1024")
                for m in range(2)
            ]
            ps_e = [mk_psum(f"pse_{m}") for m in range(2)]
            for n in range(NB):
                for k in range(4):
                    for m in range(2):
                        nc.tensor.matmul(
                            ps_e[m][:, n * 512:(n + 1) * 512],
                            w2_sb[k][:, m * 128:(m + 1) * 128],
                            h1[k][:, n * 512:(n + 1) * 512],
                            start=(k == 0),
                            stop=(k == 3),
                        )
            for m, n in [(0, 0), (1, 1)]:
                sl = slice(n * 512, (n + 1) * 512)
                nc.scalar.activation(
                    emb[m][:, sl], ps_e[m][:, sl], RELU, bias=b2_sb[:, m:m + 1]
                )
            for m, n in [(1, 0), (0, 1)]:
                sl = slice(n * 512, (n + 1) * 512)
                nc.vector.tensor_scalar(
                    emb[m][:, sl], ps_e[m][:, sl], b2_sb[:, m:m + 1], 0.0,
                    op0=ADD, op1=MAX,
                )

            # ------- experts, pass 1: primary treatments on sorted columns.
            # Expert t covers columns [O1[t], O1[t]+n1[t]); relu+bias write
            # straight into lat1. T_b is folded in via a K=1 matmul when
            # nonzero (it is all-zero for this model, so skipped).
            O1 = [0]
            for t in range(T):
                O1.append(O1[-1] + n1[t])
            lat1 = [
                xpr.tile([128, B], BF16, name=f"lat1_{f}", tag=f"lat1_{f}")
                for f in range(2)
            ]
            ps_p1 = [mk_psum(f"psp1_{f}") for f in range(2)]
            for t in range(T):
                if n1[t] == 0:
                    continue
                for f in range(2):
                    for (a, b) in _chunks512(O1[t], O1[t + 1]):
                        for k in range(2):
                            nc.tensor.matmul(
                                ps_p1[f][:, a:b],
                                tw_sb[t][:, k * N2 + f * 128: k * N2 + (f + 1) * 128],
                                emb[k][:, a:b],
                                start=(k == 0),
                                stop=(k == 1) and tb_zero,
                            )
                        if not tb_zero:
                            nc.tensor.matmul(
                                ps_p1[f][:, a:b],
                                tbr_sb[0:1, t * N2 + f * 128: t * N2 + (f + 1) * 128],
                                ones_sb[0:1, : b - a],
                                start=False,
                                stop=True,
                            )
            for f in range(2):
                for (a, b) in _chunks512(0, n_act):
                    nc.scalar.activation(lat1[f][:, a:b], ps_p1[f][:, a:b], RELU)
                if n_act < B:
                    nc.vector.memset(lat1[f][:, n_act:B], 0.0)

            # ------- pass 2a: transpose emb -> embT (emb column blocks on
            # partitions) for the gather's stationary operand
            embT = xpr.tile([128, 2048], BF16, name="embT", tag="embT")
            for half in range(2):
                trp = psp.tile([128, 1024], BF16, name=f"trp_{half}", tag="ps")
                for jj in range(4):
                    jb = half * 4 + jj
                    for k in range(2):
                        nc.tensor.transpose(
                            trp[:, jj * 256 + k * 128: jj * 256 + (k + 1) * 128],
                            emb[k][:, jb * 128:(jb + 1) * 128],
                            id_sb[:],
                        )
                for (a, b) in _chunks512(0, 1024):
                    nc.vector.tensor_copy(
                        embT[:, half * 1024 + a: half * 1024 + b], trp[:, a:b]
                    )

            # ------- pass 2b: gather secondary slots: emb_g[e, s] =
            # sum_b embT[b, e] * P[b, s] (P one-hot)
            ps_g = [mk_psum(f"psg_{e}") for e in range(2)]
            for e in range(2):
                for (a, b) in _chunks512(0, S2P):
                    for jb in range(8):
                        nc.tensor.matmul(
                            ps_g[e][:, a:b],
                            embT[:, jb * 256 + e * 128: jb * 256 + (e + 1) * 128],
                            p_sb[jb][:, a:b],
                            start=(jb == 0),
                            stop=(jb == 7),
                        )
            emb_g = [
                xpr.tile([128, S2P], BF16, name=f"embg_{e}", tag=f"embg_{e}")
                for e in range(2)
            ]
            for e in range(2):
                for (a, b) in _chunks512(0, S2P):
                    nc.scalar.activation(emb_g[e][:, a:b], ps_g[e][:, a:b], IDENT)

            # ------- pass 2c: experts on gathered slots (expert t covers
            # slot range [O[t], O[t]+cap2[t]); ranges live on the free dim so
            # no PSUM partition-alignment constraints apply), then PE-
            # transpose the relu'd output into the scatter's stationary
            # layout out2T[s, e']
            ps_p2 = [mk_psum(f"psp2_{f}") for f in range(2)]
            for t in range(T):
                if cap2[t] == 0:
                    continue
                for f in range(2):
                    for (a, b) in _chunks512(O[t], O[t + 1]):
                        for k in range(2):
                            nc.tensor.matmul(
                                ps_p2[f][:, a:b],
                                tw_sb[t][:, k * N2 + f * 128: k * N2 + (f + 1) * 128],
                                emb_g[k][:, a:b],
                                start=(k == 0),
                                stop=(k == 1) and tb_zero,
                            )
                        if not tb_zero:
                            nc.tensor.matmul(
                                ps_p2[f][:, a:b],
                                tbr_sb[0:1, t * N2 + f * 128: t * N2 + (f + 1) * 128],
                                ones_sb[0:1, : b - a],
                                start=False,
                                stop=True,
                            )
            out2 = [
                xpr.tile([128, S2P], BF16, name=f"out2_{f}", tag=f"out2_{f}")
                for f in range(2)
            ]
            for f in range(2):
                for (a, b) in _chunks512(0, S2):
                    nc.scalar.activation(out2[f][:, a:b], ps_p2[f][:, a:b], RELU)
                if S2 < S2P:
                    nc.vector.memset(out2[f][:, S2:S2P], 0.0)
            out2T = xpr.tile([128, NS2 * 256], BF16, name="out2T", tag="out2T")
            n_trh = (NS2 + 3) // 4
            for half in range(n_trh):
                j2w = min(4, NS2 - half * 4)
                trp2 = psp.tile([128, 1024], BF16, name=f"trp2_{half}", tag="ps")
                for jj in range(j2w):
                    j2 = half * 4 + jj
                    for f in range(2):
                        nc.tensor.transpose(
                            trp2[:, jj * 256 + f * 128: jj * 256 + (f + 1) * 128],
                            out2[f][:, j2 * 128:(j2 + 1) * 128],
                            id_sb[:],
                        )
                for (a, b) in _chunks512(0, j2w * 256):
                    nc.vector.tensor_copy(
                        out2T[:, half * 1024 + a: half * 1024 + b], trp2[:, a:b]
                    )

            # ------- pass 2d + dec1, chunk-pipelined: scatter-add slots back
            # to columns (lat = lat1 + out2T.T @ Q), then dec1 on each chunk
            d1 = [
                acts.tile([128, B], BF16, name=f"d1_{m}", tag="a1024")
                for m in range(4)
            ]
            lat = [
                xpr.tile([128, B], BF16, name=f"lat_{f}", tag=f"lat_{f}")
                for f in range(2)
            ]
            ps_sc = {}
            for n in range(NB):
                for f in range(2):
                    ps_sc[(n, f)] = psp.tile(
                        [128, 512], F32, name=f"pssc_{n}_{f}", tag="ps"
                    )
                    for j2 in range(NS2):
                        nc.tensor.matmul(
                            ps_sc[(n, f)][:],
                            out2T[:, j2 * 256 + f * 128: j2 * 256 + (f + 1) * 128],
                            q_sb[j2][:, n * 512:(n + 1) * 512],
                            start=(j2 == 0),
                            stop=(j2 == NS2 - 1),
                        )
            for n in range(NB):
                sl = slice(n * 512, (n + 1) * 512)
                for f in range(2):
                    nc.vector.tensor_add(
                        lat[f][:, sl], lat1[f][:, sl], ps_sc[(n, f)][:]
                    )
                ps_d1n = [
                    psp.tile([128, 512], F32, name=f"psd1_{n}_{m}", tag="ps")
                    for m in range(4)
                ]
                for m in range(4):
                    for k in range(2):
                        nc.tensor.matmul(
                            ps_d1n[m][:],
                            dw1_sb[k][:, m * 128:(m + 1) * 128],
                            lat[k][:, sl],
                            start=(k == 0),
                            stop=(k == 1),
                        )
                for m in range(4):
                    nc.scalar.activation(
                        d1[m][:, sl], ps_d1n[m][:], RELU, bias=db1_sb[:, m:m + 1]
                    )

            # ------- dec2 (bf16): [512,1024] -> [512,1024]
            d2 = [
                acts.tile([128, B], BF16, name=f"d2_{m}", tag="a1024")
                for m in range(4)
            ]
            ps_d2 = [mk_psum(f"psd2_{m}") for m in range(4)]
            for k in range(4):
                for m in range(4):
                    for n in range(NB):
                        nc.tensor.matmul(
                            ps_d2[m][:, n * 512:(n + 1) * 512],
                            dw2_sb[k][:, m * 128:(m + 1) * 128],
                            d1[k][:, n * 512:(n + 1) * 512],
                            start=(k == 0),
                            stop=(k == 3),
                        )
            # d2 epilogue split ACT/DVE (halves the serial chain gating the
            # first dec3 vars matmuls, which contract all four d2 tiles)
            for m in range(2):
                nc.scalar.activation(d2[m][:], ps_d2[m][:], RELU, bias=db2_sb[:, m:m + 1])
            for m in range(2, 4):
                for n in range(NB):
                    sl = slice(n * 512, (n + 1) * 512)
                    nc.vector.tensor_scalar(
                        d2[m][:, sl], ps_d2[m][:, sl], db2_sb[:, m:m + 1], 0.0,
                        op0=ADD, op1=MAX,
                    )
            # fp8 copy of d2 for the DoubleRow means matmuls, laid out as
            # k-pair planes per 512-col chunk (DVE from SBUF d2, NOT from
            # dec2 PSUM -- PSUM readers would stall dec3's tile rotation):
            # d2f8[:, kk*2048 + n*1024 + pl*512 + c] = d2[2kk+pl][p, n*512+c]
            d2f8 = xpr.tile([128, 4096], F8, name="d2f8", tag="d2f8")
            for m in range(4):
                kk, pl = m // 2, m % 2
                for n in range(NB):
                    nc.vector.tensor_copy(
                        d2f8[:, kk * 2048 + n * 1024 + pl * 512:
                             kk * 2048 + n * 1024 + (pl + 1) * 512],
                        d2[m][:, n * 512:(n + 1) * 512],
                    )

            # ------- dec3 + output heads (see module docstring)
            def store_pair(o, out_row0, j, q=None):
                # the last means pairs store via the (idle) Act queue so the
                # final stores drain in parallel with the Sync queue's
                q = q or nc.sync
                r0 = out_row0 + 2 * j * 128
                if j < MT_HALF // 2 - 1:
                    # both mi full: one DMA writes 256 DRAM rows
                    q.dma_start(
                        out=yt[r0:r0 + 256, :].rearrange("(t p) b -> p t b", p=128),
                        in_=o.rearrange("p (t b) -> p t b", t=2),
                    )
                else:
                    q.dma_start(out=yt[r0:r0 + 128, :], in_=o[:, :B])
                    tail = IN_F - 128 * (MT_HALF - 1)
                    q.dma_start(
                        out=yt[r0 + 128:r0 + 128 + tail, :],
                        in_=o[:tail, B:],
                    )

            def dec3_vars(j):
                w3k = w3s.tile([128, 1024], BF16, name=f"w3v_{j}", tag="w3")
                nc.gpsimd.dma_start(out=w3k[:], in_=w3v[j])
                o = outs.tile([128, 2 * B], F16, name=f"ov_{j}", tag="o")
                for mi2 in range(2):
                    mi = 2 * j + mi2
                    mw = 128 if mi < MT_HALF - 1 else (IN_F - 128 * (MT_HALF - 1))
                    ps = mk_psum(f"ps3v_{mi}")
                    for k in range(4):
                        for n in range(NB):
                            nc.tensor.matmul(
                                ps[:, n * 512:(n + 1) * 512],
                                w3k[:, mi2 * 512 + k * 128: mi2 * 512 + (k + 1) * 128],
                                d2[k][:, n * 512:(n + 1) * 512],
                                start=(k == 0),
                                stop=(k == 3),
                            )
                    osl = o[:mw, mi2 * B:(mi2 + 1) * B]
                    bias_ap = b3v_sb[:mw, mi:mi + 1]
                    # softplus+0.001 = ln(C + C*e^x); sc kept bf16 (2x DVE,
                    # ~0.25% on vars which is inside the error budget)
                    sc = rp.tile([128, B], BF16, name=f"sc_{mi}", tag="sc")
                    nc.scalar.activation(sc[:mw, :], ps[:mw, :], EXP, bias=bias_ap)
                    nc.vector.tensor_scalar(
                        sc[:mw, :], sc[:mw, :], SP_C, SP_C, op0=MULT, op1=ADD
                    )
                    nc.scalar.activation(osl, sc[:mw, :], LN)
                store_pair(o, IN_F, j)

            def dec3_means(j, q=None):
                w3k8 = w3s8.tile([128, 1024], F8, name=f"w3m_{j}", tag="w38")
                nc.gpsimd.dma_start(out=w3k8[:], in_=w3m8[j])
                o = outs.tile([128, 2 * B], F16, name=f"om_{j}", tag="o")
                for mi2 in range(2):
                    mi = 2 * j + mi2
                    mw = 128 if mi < MT_HALF - 1 else (IN_F - 128 * (MT_HALF - 1))
                    ps = mk_psum(f"ps3m_{mi}")
                    for kk in range(2):
                        for n in range(NB):
                            nc.tensor.matmul(
                                ps[:, n * 512:(n + 1) * 512],
                                w3k8[:, mi2 * 512 + kk * 256:
                                     mi2 * 512 + (kk + 1) * 256].rearrange(
                                    "p (two m) -> p two m", two=2
                                ),
                                d2f8[:, kk * 2048 + n * 1024:
                                     kk * 2048 + (n + 1) * 1024].rearrange(
                                    "p (two c) -> p two c", two=2
                                ),
                                start=(kk == 0),
                                stop=(kk == 1),
                                perf_mode=DOUBLE_ROW,
                            )
                    osl = o[:mw, mi2 * B:(mi2 + 1) * B]
                    bias_ap = b3m_sb[:mw, mi:mi + 1]
                    # undo the x128 weight scale and add bias; mi2=0 on ACT
                    # (it has slack between the vars EXP/LN chains), mi2=1 on
                    # DVE in 512 chunks so the kernel tail drains fast
                    if mi2 == 0:
                        nc.scalar.activation(
                            osl, ps[:mw, :], IDENT, bias=bias_ap,
                            scale=1.0 / W3M_SCALE,
                        )
                    else:
                        for n in range(NB):
                            sl = slice(n * 512, (n + 1) * 512)
                            nc.vector.tensor_scalar(
                                o[:mw, mi2 * B + n * 512: mi2 * B + (n + 1) * 512],
                                ps[:mw, sl], 1.0 / W3M_SCALE, bias_ap,
                                op0=MULT, op1=ADD,
                            )
                store_pair(o, 0, j, q=q)

            # order: v0 v1 m0 v2 m1 ... m17 v19 m18 m19
            dec3_vars(0)
            dec3_vars(1)
            for j in range(2, MT_HALF // 2):
                dec3_means(j - 2)
                dec3_vars(j)
            dec3_means(MT_HALF // 2 - 2)
            dec3_means(MT_HALF // 2 - 1)

    split_sync_waits(nc)
    return nc


# ---------------------------------------------------------------------------
# Host glue
# ---------------------------------------------------------------------------
_NC_CACHE = {}


def _get_nc(route):
    key = route.meta
    if key not in _NC_CACHE:
        _NC_CACHE[key] = build_bass(key)
    return _NC_CACHE[key]


def _bias_cols(b, ntiles):
    """[D] -> [128, ntiles]; col j = b[j*128:(j+1)*128], zero-padded."""
    out = np.zeros((128, ntiles), np.float32)
    b = np.asarray(b, np.float32)
    for j in range(ntiles):
        seg = b[j * 128:min((j + 1) * 128, b.shape[0])]
        out[: seg.shape[0], j] = seg
    return out


def _prep_shared(inputs, route):
    f32 = lambda a: np.ascontiguousarray(np.asarray(a), dtype=np.float32)
    bf16 = ml_dtypes.bfloat16
    f8 = ml_dtypes.float8_e4m3
    w1 = f32(inputs["enc_W1"])
    w2 = f32(inputs["enc_W2"])
    tw = f32(inputs["T_W"])
    dw1 = f32(inputs["dec_W1"])
    dw2 = f32(inputs["dec_W2"])
    w3 = f32(inputs["dec_W3"])

    # w1 zero-padded to [5120, 512], packed in pairs of k-tiles:
    # w1p[j, p, s*512 + c] = W1[(2j+s)*128 + p, c]
    w1z = np.zeros((IN_FP, N0), np.float32)
    w1z[:IN_F] = w1
    w1p = np.ascontiguousarray(
        w1z.reshape(KP, 2, 128, N0).transpose(0, 2, 1, 3).reshape(KP, 128, 2 * N0)
    ).astype(bf16)

    # T_W packed: twp[t, p, k*256 + c] = T_W[t, k*128 + p, c]
    twp = np.ascontiguousarray(
        tw.reshape(T, 2, 128, N2).transpose(0, 2, 1, 3).reshape(T, 128, 2 * N2)
    ).astype(bf16)

    # dec_W3 vars half (bf16) packed in mi-pairs:
    # w3p[j, p, mi2*512 + k*128 + c] = W3[k*128 + p, (2j+mi2)*128 + c]
    def tile_w3(cols):
        out = np.zeros((MT_HALF // 2, 128, 1024), np.float32)
        for k in range(4):
            blk = cols[k * 128:(k + 1) * 128, :]          # [128, <=5120]
            cw = blk.shape[1]
            padded = np.zeros((128, MT_HALF * 128), np.float32)
            padded[:, :cw] = blk
            per_mi = padded.reshape(128, MT_HALF, 128).transpose(1, 0, 2)
            for mi2 in range(2):
                out[:, :, mi2 * 512 + k * 128: mi2 * 512 + (k + 1) * 128] = (
                    per_mi[mi2::2]
                )
        return np.ascontiguousarray(out).astype(bf16)

    w3v = tile_w3(w3[:, IN_F:])

    # dec_W3 means half (fp8e4 x128) with DoubleRow k-pair planes:
    # w3m8[j, p, mi2*512 + kk*256 + pl*128 + c]
    #   = 128 * W3[(2kk+pl)*128 + p, (2j+mi2)*128 + c]
    def tile_w3m8(cols):
        out = np.zeros((MT_HALF // 2, 128, 1024), np.float32)
        for k in range(4):
            kk, pl = k // 2, k % 2
            blk = cols[k * 128:(k + 1) * 128, :]
            cw = blk.shape[1]
            padded = np.zeros((128, MT_HALF * 128), np.float32)
            padded[:, :cw] = blk
            per_mi = padded.reshape(128, MT_HALF, 128).transpose(1, 0, 2)
            for mi2 in range(2):
                out[:, :, mi2 * 512 + kk * 256 + pl * 128:
                    mi2 * 512 + kk * 256 + (pl + 1) * 128] = per_mi[mi2::2]
        return np.ascontiguousarray(out * W3M_SCALE).astype(f8)

    w3m8 = tile_w3m8(w3[:, :IN_F])

    shared = {
        "w1p": w1p,
        "w2": w2.astype(bf16),
        "twp": twp,
        "dw1": dw1.astype(bf16),
        "dw2": dw2.astype(bf16),
        "w3m8": w3m8,
        "w3v": w3v,
        "idm": np.eye(128, dtype=np.float32).astype(bf16),
        "tbr": np.ascontiguousarray(
            np.asarray(inputs["T_b"], np.float32).reshape(1, T * N2)
        ).astype(bf16),
        "b1c": _bias_cols(inputs["enc_b1"], 4),
        "b2c": _bias_cols(inputs["enc_b2"], 2),
        "tbc": np.ascontiguousarray(
            np.asarray(inputs["T_b"], dtype=np.float32)
            .reshape(T, 2, 128)
            .transpose(2, 0, 1)
            .reshape(128, T * 2)
        ),
        "db1c": _bias_cols(inputs["dec_b1"], 4),
        "db2c": _bias_cols(inputs["dec_b2"], 4),
        "b3mc": _bias_cols(np.asarray(inputs["dec_b3"])[:IN_F], MT_HALF),
        "b3vc": _bias_cols(np.asarray(inputs["dec_b3"])[IN_F:], MT_HALF),
    }
    x = f32(inputs["input"])
    in_maps = []
    for c in range(N_CORES):
        m = dict(shared)
        # xT zero-padded to [5120, B] with host-permuted (routed) columns,
        # packed in pairs of k-tiles: xp[j, p, s*B + c] = xT[(2j+s)*128+p, c]
        xt = np.zeros((IN_FP, B), np.float32)
        xt[:IN_F] = x[route.perm[c], :].T
        m["xp"] = np.ascontiguousarray(
            xt.reshape(KP, 2, 128, B).transpose(0, 2, 1, 3).reshape(KP, 128, 2 * B)
        ).astype(bf16)
        m["pd"] = np.ascontiguousarray(
            route.P[c].reshape(8, 128, route.S2P)
        ).astype(bf16)
        qpad = np.zeros((route.NS2 * 128, B), np.float32)
        qpad[: route.S2P] = route.Q[c]
        m["qd"] = np.ascontiguousarray(
            qpad.reshape(route.NS2, 128, B)
        ).astype(bf16)
        in_maps.append(m)
    return in_maps


def kernel(**inputs) -> np.ndarray:
    route = _route(inputs)
    nc = _get_nc(route)
    in_maps = _prep_shared(inputs, route)
    res = run_bass_kernel_spmd(nc, in_maps, core_ids=list(range(N_CORES)))
    out = np.empty((BATCH, 2 * IN_F), np.float32)
    for c in range(N_CORES):
        out[route.perm[c], :] = res.results[c]["yt"].T.astype(np.float32)
    return out
